# revision 36
# baseline (speedup 1.0000x reference)
"""AttentionAugmentedConv2d Trainium2 Bass kernel.

Data-parallel over batch: 8 samples -> 8 NeuronCores, one sample per core.
Self-contained: hardcodes all shapes; builds derived constant inputs on host.

v3: 16-bit matmul datapath (fp32 streams at ~2 cycles/column on TRN2;
fp16/bf16 stream at 1), engine rebalancing, rel tables on the idle logits
PSUM tag, natural-order q copy (kills strided-rhs matmuls), 2-ahead head
prep, conv/attention interleave tuned to keep the PE HAM-warm.

Per-core pipeline (sample = x (256, 32, 32)):
  1. 3x3 convs as 9-tap matmul accumulation in fp16 (x and weights cast on
     host; x pre-padded to 34x34 on host). Epilogues (bias add) on ScalarE
     while it is exp-idle (first 3 blocks) else DVE; q is stored both in
     m'-order (col = 32*c + r) and naturally, k fp16, v bf16.
  2. v -> v^T via PE transpose (bf16) into one [128, 4608] block tile
     [v_h(32) | ones(32)] x 9 per u-tile; ones rows give softmax sums free.
  3. Relative tables rwT/rhT = key_rel^T @ q_h as fp16 matmuls against a
     host-prebuilt diagonal key_rel image; skewed via a DRAM bounce (fp16)
     into A_w (m'-store) and A_h (natural-store).
  4. Per head: logitsT (n, m') in one K=96 fp16 matmul
     [k; I_w; I_h] x [q; A_w; A_h']; exp on ScalarE -> bf16 (no max
     subtraction); att^T + sums accumulated in bf16 matmuls vs the vT block.
  5. DVE 32x32 block transposes convert att^T (d, m') into the
     torch-faithful scrambled (channel, spatial) layout; per-head
     approx-reciprocal + multiply normalizes -> bf16; 1x1 conv in bf16;
     concat with xo via output row ranges. Output f32.
"""

import sys

sys.path.insert(0, "/opt/trn_rl_repo")

import numpy as np

import concourse.bass as bass
import concourse.tile as tile
from concourse import bacc, mybir
from concourse.bass_types import AP

F32 = mybir.dt.float32
F16 = mybir.dt.float16
BF16 = mybir.dt.bfloat16

B, CIN, COUT, DK, DV, NH, H, W = 8, 256, 512, 256, 256, 8, 32, 32
DKH = DK // NH  # 32
HW = H * W  # 1024
SCALE = DKH ** -0.5
N_CORES = 8

_cached = {}


def build_bass():
    nc = bacc.Bacc("TRN2", target_bir_lowering=False, debug=False,
                   num_devices=N_CORES)

    # x pre-padded to 34x34 on host, fp16
    x_d = nc.dram_tensor("xpad", [2, 128, 34 * 34], F16,
                         kind="ExternalInput").ap()
    # conv weights: (cout_tile, ci_half, ci_local, tap, co_local) fp16;
    # qkv tiles 0-5 (q0 q1 k0 k1 v0 v1), conv tiles 6-7
    wq_d = nc.dram_tensor("wqkvT", [6, 2, 128, 9, 128], F16,
                          kind="ExternalInput").ap()
    wc_d = nc.dram_tensor("wconvT", [2, 2, 128, 9, 128], F16,
                          kind="ExternalInput").ap()
    wattT_d = nc.dram_tensor("wattT", [2, 128, 256], BF16,
                             kind="ExternalInput").ap()
    bcol_d = nc.dram_tensor("bias_cols", [128, 12], F32,
                            kind="ExternalInput").ap()
    # key_rel diagonal image: [p, (t, a, c)] with krT at rows 32a, cols 0:63
    krc_d = nc.dram_tensor("krc_img", [128, 1024], F16,
                           kind="ExternalInput").ap()
    # [I_w ; I_h] rows for the fused logits lhsT
    ltc_d = nc.dram_tensor("lt_const", [64, HW], F16,
                           kind="ExternalInput").ap()
    i128_d = nc.dram_tensor("I128", [128, 128], BF16,
                            kind="ExternalInput").ap()
    vinit_d = nc.dram_tensor("vinit", [4608], BF16,
                             kind="ExternalInput").ap()
    out_d = nc.dram_tensor("out", [COUT, HW], F32, kind="ExternalOutput").ap()

    with tile.TileContext(nc) as tc:
        _build(nc, tc, x_d, wq_d, wc_d, wattT_d, bcol_d, krc_d, ltc_d,
               i128_d, vinit_d, out_d)
    nc.compile()
    return nc


def _build(nc, tc, x_d, wq_d, wc_d, wattT_d, bcol_d, krc_d, ltc_d,
           i128_d, vinit_d, out_d):
    from contextlib import ExitStack

    ctx = ExitStack()
    with ctx:
        persist = ctx.enter_context(tc.tile_pool(name="persist", bufs=1))
        wqk_pool = ctx.enter_context(tc.tile_pool(name="wqk", bufs=16))
        e_pool = ctx.enter_context(tc.tile_pool(name="epool", bufs=8))
        sb_pool = ctx.enter_context(tc.tile_pool(name="sbp", bufs=4))
        dram_pool = ctx.enter_context(
            tc.tile_pool(name="dramp", bufs=1, space="DRAM"))
        # PSUM: cp(2 banks) + lp(4) + ap(2) = 8 banks
        cpsum = ctx.enter_context(
            tc.tile_pool(name="cpsum", bufs=2, space="PSUM"))
        lpsum = ctx.enter_context(
            tc.tile_pool(name="lpsum", bufs=2, space="PSUM"))
        apsum = ctx.enter_context(
            tc.tile_pool(name="apsum", bufs=2, space="PSUM"))

        # ---------- persistent SBUF ----------
        xpad = [persist.tile([128, 34 * 34], F16, tag=f"xpad{i}",
                             name=f"xpad{i}") for i in range(2)]
        q_st = [persist.tile([128, HW], F16, tag=f"qst{i}", name=f"qst{i}")
                for i in range(2)]
        q_nat = [persist.tile([128, HW], F16, tag=f"qnat{i}",
                              name=f"qnat{i}") for i in range(2)]
        k_st = [persist.tile([128, HW], F16, tag=f"kst{i}", name=f"kst{i}")
                for i in range(2)]
        v_st = [persist.tile([128, HW], BF16, tag=f"vst{i}", name=f"vst{i}")
                for i in range(2)]
        vT = persist.tile([128, 4608], BF16, tag="vT", name="vT")
        i128 = persist.tile([128, 128], BF16, tag="i128", name="i128")
        wattT_sb = [persist.tile([128, 256], BF16, tag=f"watt{i}",
                                 name=f"watt{i}") for i in range(2)]
        bcol = persist.tile([128, 12], F32, tag="bcol", name="bcol")
        krc = persist.tile([128, 1024], F16, tag="krc", name="krc")
        att_un = [persist.tile([128, HW], F32, tag=f"attun{g}",
                               name=f"attun{g}") for g in range(2)]
        smap = [persist.tile([128, HW], F32, tag=f"smap{g}", name=f"smap{g}")
                for g in range(2)]
        rmap = [persist.tile([128, HW], F32, tag=f"rmap{g}", name=f"rmap{g}")
                for g in range(2)]
        attn = [persist.tile([128, HW], BF16, tag=f"attn{g}", name=f"attn{g}")
                for g in range(2)]
        rscr = persist.tile([128, HW], F32, tag="rscr", name="rscr")

        # logits operand rings: lt rows 0:32 k_h, 32:96 const [I_w; I_h];
        # rt rows 0:32 q_h, 32:64 A_w (m'), 64:96 A_h (m'-ified)
        lt_s = [persist.tile([96, HW], F16, tag=f"lts{s}", name=f"lts{s}")
                for s in range(3)]
        rt_s = [persist.tile([96, HW], F16, tag=f"rts{s}", name=f"rts{s}")
                for s in range(3)]
        skw = [[dram_pool.tile([94 * HW], F16, tag=f"skw{h}_{t}",
                               name=f"skw{h}_{t}") for t in range(2)]
               for h in range(NH)]

        # ---------- loads: x + weights on sync (conv critical path),
        # constants on gpsimd; ScalarE issues no DMA ----------
        nc.gpsimd.dma_start(bcol[:], bcol_d[:])
        nc.sync.dma_start(xpad[0][:], x_d[0])
        nc.scalar.dma_start(xpad[1][:], x_d[1])
        nc.gpsimd.dma_start(krc[:], krc_d[:])
        for s in range(3):
            nc.gpsimd.dma_start(lt_s[s][32:96, :], ltc_d[:])
        nc.gpsimd.dma_start(
            vT[:], AP(vinit_d.tensor, 0, [[0, 128], [1, 4608]]))
        nc.gpsimd.dma_start(i128[:], i128_d[:])
        for i in range(2):
            nc.gpsimd.dma_start(wattT_sb[i][:], wattT_d[i])

        def xwin(cih, r0, nr, dy, dx):
            v = xpad[cih][:].rearrange("p (a b) -> p a b", a=34)
            return v[:, dy + r0: dy + r0 + nr, dx:dx + 32]

        # ---------- q,k,v + xo convs (fp16) ----------
        def conv_feed(w_src, epilogue, w_eng=None, ws=None):
            # loads weights eagerly, returns a generator that emits the 36
            # matmuls in 18 two-tap chunks (yield between chunks) so conv
            # work can be threaded between attention matmuls
            if ws is None:
                ws = []
                for c in range(2):
                    wt = wqk_pool.tile([128, 9 * 128], F16, tag="wqk",
                                       name="wqk")
                    (w_eng or nc.sync).dma_start(
                        wt[:], w_src[c].rearrange("p t co -> p (t co)"))
                    ws.append(wt)

            def gen():
                for st in range(2):
                    cps = cpsum.tile([128, 512], F32, tag="c", name="cps")
                    i = 0
                    for t in range(9):
                        dy, dx = t // 3, t % 3
                        for c in range(2):
                            nc.tensor.matmul(
                                cps[:], ws[c][:, 128 * t:128 * (t + 1)],
                                xwin(c, 16 * st, 16, dy, dx),
                                start=(i == 0), stop=(i == 17))
                            i += 1
                        yield
                    epilogue(st, cps)
            return gen()

        def conv_block(w_src, epilogue, w_eng=None, ws=None):
            for _ in conv_feed(w_src, epilogue, w_eng, ws=ws):
                pass

        fillers = []

        def fill(n):
            while n > 0 and fillers:
                try:
                    next(fillers[0])
                    n -= 1
                except StopIteration:
                    fillers.pop(0)

        def drain_fillers():
            while fillers:
                try:
                    next(fillers[0])
                except StopIteration:
                    fillers.pop(0)

        CP = mybir.ActivationFunctionType.Identity

        def qkv_epi(cc, on_scalar):
            # bias add + dtype cast out of conv PSUM.  q (cc<2) is written
            # twice: m'-order into q_st and naturally into q_nat (bias col
            # pre-scaled by SCALE on host for the scalar-engine form).
            def epi(st, cps, cc=cc):
                b = bcol[:, cc:cc + 1]
                if cc < 2:
                    qv = q_st[cc][:].rearrange("p (c r) -> p r c", r=32)
                    qm = qv[:, 16 * st:16 * (st + 1), :]
                    cv = cps[:].rearrange("p (r c) -> p r c", r=16)
                    qn = q_nat[cc][:, 512 * st:512 * (st + 1)]
                    bs = bcol[:, 10 + cc:11 + cc]
                    nc.vector.tensor_scalar(
                        qm, cv, b, SCALE, mybir.AluOpType.add,
                        mybir.AluOpType.mult)
                    if on_scalar:
                        nc.scalar.activation(qn, cps[:], CP, bias=bs,
                                             scale=SCALE)
                    else:
                        nc.vector.tensor_scalar(
                            qn, cps[:], b, SCALE, mybir.AluOpType.add,
                            mybir.AluOpType.mult)
                else:
                    dst = (k_st[cc - 2] if cc < 4 else v_st[cc - 4])
                    dv = dst[:, 512 * st:512 * (st + 1)]
                    if on_scalar:
                        nc.scalar.activation(dv, cps[:], CP, bias=b)
                    else:
                        nc.vector.tensor_scalar(
                            dv, cps[:], b, None, mybir.AluOpType.add)
            return epi

        def xo_epi(cc):
            def epi(st, cps, cc=cc):
                osb = sb_pool.tile([128, 512], F32, tag="osb", name="osb")
                nc.vector.tensor_scalar(
                    osb[:], cps[:], bcol[:, 6 + cc:7 + cc], None,
                    mybir.AluOpType.add)
                nc.sync.dma_start(
                    out_d[128 * cc:128 * (cc + 1), 512 * st:512 * (st + 1)],
                    osb[:])
            return epi

        # ---------- v -> v^T via PE transpose (bf16) ----------
        def vtrans(half):
            vt = lpsum.tile([128, 1024], BF16, tag="l", name="vt")
            for u in range(8):
                nc.tensor.transpose(
                    vt[:, 128 * u:128 * u + 128],
                    v_st[half][:, 128 * u:128 * (u + 1)], i128[:])
            for u in range(8):
                dst = vT[:, 576 * u:576 * (u + 1)].rearrange(
                    "p (h j) -> p h j", h=9)
                nc.vector.tensor_copy(
                    dst[:, 4 * half:4 * (half + 1), 0:32],
                    vt[:, 128 * u:128 * (u + 1)].rearrange(
                        "p (h d) -> p h d", h=4))

        # ---------- relative tables + skew bounce (fp16, lp-tag PSUM) ----
        # tab 0 (rw): m'-order stream; slot 1024*j + 1056*c + r,
        #   read A_w[p,(c,r)] at 1024*(p+31) + 32*c + r (m'-stored)
        # tab 1 (rh): natural stream; slot 1024*j + 1056*r + c,
        #   read A_h[p,(r,c)] at 1024*(p+31) + 32*r + c (natural-stored)
        def rel_head(h):
            qt, j4 = h // 4, h % 4
            for tab in range(2):
                rps = lpsum.tile([128, 1024], F32, tag="l", name="rps")
                for st in range(2):
                    qrhs = (q_st if tab == 0 else q_nat)[qt][
                        :, 512 * st:512 * (st + 1)]
                    nc.tensor.matmul(
                        rps[:, 512 * st:512 * (st + 1)],
                        krc[:, 512 * tab + 128 * j4:
                            512 * tab + 128 * (j4 + 1)],
                        qrhs, start=True, stop=True)
                rsb = sb_pool.tile([64, 1024], F16, tag="rsb", name="rsb")
                nc.vector.tensor_copy(rsb[0:63, :], rps[0:63, :])
                # dense write at row stride 1056; the diagonal skew happens
                # on the read side via a negative partition stride
                dst = AP(skw[h][tab].tensor, 0, [[1056, 63], [1, 1024]])
                nc.gpsimd.dma_start(dst, rsb[0:63, :])

        # ---------- attention head ----------
        ap_t = {}

        def head_prep(h):
            qt, j4, s = h // 4, h % 4, h % 3
            lt, rt = lt_s[s], rt_s[s]
            nc.vector.tensor_copy(lt[0:32, :],
                                  k_st[qt][32 * j4:32 * j4 + 32, :])
            nc.vector.tensor_copy(rt[0:32, :],
                                  q_st[qt][32 * j4:32 * j4 + 32, :])
            skr = AP(skw[h][0].tensor, 31 * 1056,
                     [[1056, 32], [-1024, 32], [1, 32]])
            nc.sync.dma_start(
                rt[32:64, :].rearrange("p (a b) -> p a b", a=32), skr)
            ah = sb_pool.tile([32, HW], F16, tag="ah", name="ah")
            skr = AP(skw[h][1].tensor, 31 * 1056,
                     [[1056, 32], [-1024, 32], [1, 32]])
            nc.sync.dma_start(
                ah[:].rearrange("p (a b) -> p a b", a=32), skr)
            nc.vector.tensor_copy(
                rt[64:96, :].rearrange("p (c r) -> p c r", c=32),
                ah[:].rearrange("p (r c) -> p c r", c=32))

        def head_mms(h, per_u_fill=1):
            # software-pipelined: att(u-1) is emitted after logits(u) so the
            # PE never waits on exp(u); filler conv chunks pace the loop
            s = h % 3
            lt, rt = lt_s[s], rt_s[s]
            ap0 = apsum.tile([128, 512], F32, tag="a", name="ap0")
            ap1 = apsum.tile([128, 512], F32, tag="a", name="ap1")
            ap_t[h] = (ap0, ap1)

            def att(u, e_u):
                nc.tensor.matmul(ap0[:],
                                 vT[:, 576 * u + 64 * h:
                                    576 * u + 64 * h + 128],
                                 e_u[:, 0:512],
                                 start=(u == 0), stop=(u == 7))
                nc.tensor.matmul(ap1[:],
                                 vT[:, 576 * u + 64 * h:
                                    576 * u + 64 * h + 128],
                                 e_u[:, 512:HW],
                                 start=(u == 0), stop=(u == 7))

            prev = None
            for u in range(8):
                lps = lpsum.tile([128, HW], F32, tag="l", name="lps")
                for mh in range(2):
                    nc.tensor.matmul(
                        lps[:, 512 * mh:512 * (mh + 1)],
                        lt[:, 128 * u:128 * (u + 1)],
                        rt[:, 512 * mh:512 * (mh + 1)],
                        start=True, stop=True)
                e_u = e_pool.tile([128, HW], BF16, tag="E", name="E")
                nc.scalar.activation(e_u[:], lps[:],
                                     mybir.ActivationFunctionType.Exp)
                if prev is not None:
                    att(*prev)
                prev = (u, e_u)
                fill(per_u_fill)
            att(*prev)

        def head_tail(h):
            ap0, ap1 = ap_t.pop(h)
            g, po = h // 4, 32 * (h % 4)
            nc.vector.transpose(att_un[g][po:po + 32, 0:512], ap0[0:32, :])
            nc.vector.transpose(att_un[g][po:po + 32, 512:HW], ap1[0:32, :])
            nc.vector.transpose(smap[g][po:po + 32, 0:512], ap0[32:64, :])
            nc.vector.transpose(smap[g][po:po + 32, 512:HW], ap1[32:64, :])

        def norm(g):
            nc.vector.reciprocal_approx_accurate(
                rmap[g][:], smap[g][:], rscr[:])
            nc.vector.tensor_tensor(attn[g][:], att_un[g][:], rmap[g][:],
                                    mybir.AluOpType.mult)

        # ---------- emission schedule ----------
        ws_q0 = [wqk_pool.tile([128, 9 * 128], F16, tag="wqk", name="wqk")
                 for _ in range(2)]
        nc.sync.dma_start(
            ws_q0[0][:], wq_d[0][0].rearrange("p t co -> p (t co)"))
        nc.scalar.dma_start(
            ws_q0[1][:], wq_d[0][1].rearrange("p t co -> p (t co)"))
        conv_block(wq_d[0], qkv_epi(0, True), ws=ws_q0)   # q0
        conv_block(wq_d[2], qkv_epi(2, True))    # k0
        for h in range(4):
            rel_head(h)
        conv_block(wq_d[4], qkv_epi(4, True))    # v0
        vtrans(0)
        head_prep(0)
        head_prep(1)

        fillers.append(conv_feed(wq_d[1], qkv_epi(1, False)))   # q1
        fillers.append(conv_feed(wq_d[3], qkv_epi(3, False)))   # k1
        fillers.append(conv_feed(wq_d[5], qkv_epi(5, False)))   # v1
        head_mms(0, per_u_fill=2)
        head_prep(2)
        head_tail(0)
        head_mms(1, per_u_fill=2)
        rel_head(4)
        rel_head(5)
        head_prep(3)
        head_tail(1)
        head_mms(2, per_u_fill=2)
        rel_head(6)
        rel_head(7)
        head_prep(4)
        head_tail(2)
        drain_fillers()                          # finish v1
        vtrans(1)
        fillers.append(conv_feed(wc_d[0], xo_epi(0)))
        head_mms(3, per_u_fill=1)
        head_prep(5)
        head_tail(3)
        norm(0)
        fillers.append(conv_feed(wc_d[1], xo_epi(1)))
        head_mms(4, per_u_fill=1)
        head_prep(6)
        head_tail(4)
        head_mms(5, per_u_fill=1)
        head_prep(7)
        head_tail(5)
        head_mms(6, per_u_fill=1)
        head_tail(6)
        head_mms(7, per_u_fill=1)
        drain_fillers()

        # ---------- head 7 tail + 1x1 conv, split by m' halves; the kc=0
        # accumulation on attn[0] (ready since head 3) runs into freed
        # logits PSUM banks immediately, overlapping the norm chain ------
        ap0, ap1 = ap_t.pop(7)
        lpF = []
        for ct in range(2):
            lf = lpsum.tile([128, HW], F32, tag="l", name="lpf")
            lpF.append(lf)
            for st in range(2):
                nc.tensor.matmul(
                    lf[:, 512 * st:512 * (st + 1)],
                    wattT_sb[0][:, 128 * ct:128 * (ct + 1)],
                    attn[0][:, 512 * st:512 * (st + 1)],
                    start=True, stop=False)
        for st, ap in ((0, ap0), (1, ap1)):
            cs = slice(512 * st, 512 * (st + 1))
            nc.vector.transpose(att_un[1][96:128, cs], ap[0:32, :])
            nc.vector.transpose(smap[1][96:128, cs], ap[32:64, :])
            nc.vector.reciprocal_approx_accurate(
                rmap[1][:, cs], smap[1][:, cs], rscr[:, cs])
            nc.vector.tensor_tensor(attn[1][:, cs], att_un[1][:, cs],
                                    rmap[1][:, cs], mybir.AluOpType.mult)
            for ct in range(2):
                nc.tensor.matmul(
                    lpF[ct][:, cs],
                    wattT_sb[1][:, 128 * ct:128 * (ct + 1)],
                    attn[1][:, cs],
                    start=False, stop=True)
                osb = sb_pool.tile([128, 512], F32, tag="osb", name="osb")
                nc.scalar.activation(osb[:], lpF[ct][:, cs], CP,
                                     bias=bcol[:, 8 + ct:9 + ct])
                eng = nc.sync if ct == 0 else nc.gpsimd
                eng.dma_start(
                    out_d[256 + 128 * ct:256 + 128 * (ct + 1),
                          512 * st:512 * (st + 1)],
                    osb[:])


def _host_inputs(x, w_conv, b_conv, w_qkv, b_qkv, w_att, b_att,
                 key_rel_w, key_rel_h):
    """Build per-core input maps (host-side layout prep only)."""
    import ml_dtypes
    BF = ml_dtypes.bfloat16
    x = np.asarray(x, dtype=np.float32)
    xp = np.zeros((B, 2, 128, 34, 34), np.float16)
    xp[:, :, :, 1:33, 1:33] = x.reshape(B, 2, 128, 32, 32)
    xp = xp.reshape(B, 2, 128, 34 * 34)

    def wT(w, nt):
        # (co, ci, 3, 3) -> (cout_tile, ci_half, ci_local, tap, co_local)
        w = np.asarray(w, dtype=np.float32).reshape(nt, 128, 2, 128, 9)
        return np.ascontiguousarray(
            w.transpose(0, 2, 3, 4, 1)).astype(np.float16)

    wqkvT = wT(w_qkv, 6)
    wconvT = wT(w_conv, 2)
    wattT = np.ascontiguousarray(
        np.asarray(w_att, dtype=np.float32)[:, :, 0, 0].T.reshape(
            2, 128, 256)).astype(BF)
    # cols 0-5 raw qkv biases, 6-7 conv, 8-9 att, 10-11 q biases
    # pre-scaled by SCALE (ScalarE epilogue computes in*SCALE + bias)
    bias_cols = np.zeros((128, 12), np.float32)
    bias_cols[:, 0:6] = np.asarray(b_qkv, np.float32).reshape(6, 128).T
    bias_cols[:, 6:8] = np.asarray(b_conv, np.float32).reshape(2, 128).T
    bias_cols[:, 8:10] = np.asarray(b_att, np.float32).reshape(2, 128).T
    bias_cols[:, 10:12] = bias_cols[:, 0:2] * SCALE
    krc_img = np.zeros((128, 2, 4, 128), np.float16)
    for t, kr in ((0, key_rel_w), (1, key_rel_h)):
        krT = np.asarray(kr, np.float32).T.astype(np.float16)  # (32, 63)
        for a in range(4):
            krc_img[32 * a:32 * (a + 1), t, a, 0:63] = krT
    krc_img = krc_img.reshape(128, 1024)
    n = np.arange(HW)
    lt_const = np.zeros((64, HW), np.float16)
    lt_const[0:32] = (np.arange(32)[:, None] == (n % 32)[None, :])
    lt_const[32:64] = (np.arange(32)[:, None] == (n // 32)[None, :])
    I128 = np.eye(128, dtype=np.float32).astype(BF)
    vinit = np.tile(np.concatenate([
        np.zeros(32, np.float32), np.ones(32, np.float32)]), 9 * 8).astype(BF)
    shared = {
        "wqkvT": wqkvT, "wconvT": wconvT, "wattT": wattT,
        "bias_cols": bias_cols,
        "krc_img": krc_img, "lt_const": lt_const,
        "I128": I128, "vinit": vinit,
    }
    return [dict(shared, xpad=np.ascontiguousarray(xp[i]))
            for i in range(B)]


def get_nc():
    if "nc" not in _cached:
        _cached["nc"] = build_bass()
    return _cached["nc"]


def kernel(x, w_conv, b_conv, w_qkv, b_qkv, w_att, b_att,
           key_rel_w, key_rel_h):
    from concourse.bass_utils import run_bass_kernel_spmd
    nc = get_nc()
    in_maps = _host_inputs(x, w_conv, b_conv, w_qkv, b_qkv, w_att, b_att,
                           key_rel_w, key_rel_h)
    res = run_bass_kernel_spmd(nc, in_maps, list(range(N_CORES)))
    out = np.stack([res.results[i]["out"].reshape(COUT, H, W)
                    for i in range(B)])
    return out
